# revision 26
# baseline (speedup 1.0000x reference)
"""LocalBandSimilarityBlock — 8-way sequence-parallel Bass/Tile kernel for TRN2.

Sharding: rows sorted by grid-x; each of the 8 cores owns 768 consecutive
sorted query rows and a 1280-row candidate-key slab.  Each 128-query tile
attends only to a static WIN-wide window of the slab (woff_t = t*128),
verified on the host to cover every true radius-2 neighbour (sorted order
makes candidate sets contiguous; the slab start lo_c is chosen per core to
make one static window schedule fit all cores, with masked padding rows
beyond the array edges).  The exact (radius-2, no-self) mask is an additive
-1e30 bias computed on host; isolated rows attend to themselves, matching
the reference's `out = v[i]` fallback exactly.  LayerNorm-1 and the cosine
row norms are host-precomputed (O(N*D) elementwise); all matmul-heavy work
— QKV projections, the fused q@k^T + cosine-gram logits (one PSUM chain
over the stacked [q|z] contraction), softmax, attn@v (output-transposed),
Wo, LayerNorm-2, and the FFN — runs on the NeuronCores in fp8/bf16 with
f32 accumulation (fp8 DoubleRow matmuls for the projections, logits, and
FFN).  No collectives.
"""
import json
import os
import sys

import numpy as np

for _p in ("/opt/trn_rl_repo", "/opt/pypackages"):
    if os.path.isdir(_p) and _p not in sys.path:
        sys.path.append(_p)

import ml_dtypes  # noqa: E402
import concourse.bass as bass  # noqa: E402
import concourse.tile as tile  # noqa: E402
from concourse import bass2jax, bass_utils, mybir  # noqa: E402
from concourse.masks import make_identity  # noqa: E402
from concourse.tile import add_dep_helper  # noqa: E402

BF16 = mybir.dt.bfloat16
F8 = mybir.dt.float8e4
F32 = mybir.dt.float32
DR = mybir.MatmulPerfMode.DoubleRow
AF = mybir.ActivationFunctionType
ALU = mybir.AluOpType
AX = mybir.AxisListType

N, D, NCORES = 6144, 512, 8
RQ = N // NCORES  # 768 query rows per core
KC = 1280         # candidate slab per core
PAD = 512         # masked padding rows past each array edge
QT = RQ // 128    # 6
KT = KC // 128    # 10
DT = D // 128     # 4
F1 = 4 * D        # 2048
FT = F1 // 128    # 16
QCH = 384
RADIUS = 2
NEGINF = -1e30
LN_EPS = 1e-5
COS_EPS = 1e-8
WIN_LADDER = (640, 768, 896, 1024, 1280)

LAST_EXEC_NS = None
LAST_RESULTS = None

# ---------------------------------------------------------------------------
# This container's walrus rejects instructions carrying multiple sem waits
# ("Too many sync wait commands"); hoist excess waits onto single-wait NoOps.
if not getattr(bass_utils, "_drain_wait_patch", False):
    _orig_compile_bir = bass_utils.compile_bir_kernel

    def _compile_bir_patched(bir_json, tmpdir, neff_name="file.neff", **kw):
        bir = json.loads(bir_json)
        for fn in bir["functions"]:
            for blk in fn["blocks"]:
                insts = []
                for ins in blk["instructions"]:
                    si = ins.get("sync_info") or {}
                    waits = si.get("on_wait") or []
                    keep = 0 if ins.get("opcode") == "Drain" else 1
                    if len(waits) > keep:
                        cut = len(waits) - keep
                        for i, w in enumerate(waits[:cut]):
                            insts.append({
                                "debug": ins.get("debug", 0),
                                "engine": ins["engine"],
                                "ins": [], "outs": [],
                                "name": f"{ins['name']}-dw{i}",
                                "opcode": "NoOp",
                                "sync_info": {"on_update": [], "on_wait": [w]},
                            })
                        si["on_wait"] = waits[cut:]
                        ins["sync_info"] = si
                    insts.append(ins)
                blk["instructions"] = insts
        return _orig_compile_bir(json.dumps(bir).encode(), tmpdir, neff_name, **kw)

    bass_utils.compile_bir_kernel = _compile_bir_patched
    bass2jax.compile_bir_kernel = _compile_bir_patched
    bass_utils._drain_wait_patch = True


def _install_ntff_hook():
    import types
    if "antenv.axon_hooks" in sys.modules:
        return
    try:
        from trn_agent_boot.trn_boot import _ntff_profile_via_ctypes
        hook = _ntff_profile_via_ctypes("/opt/axon/libaxon_pjrt.so")
    except Exception:
        hook = None
    m = types.ModuleType("antenv.axon_hooks")
    m.get_axon_ntff_profile_hook = lambda: hook
    sys.modules["antenv.axon_hooks"] = m


def _woff(t, win):
    return min(t * 128, KC - win)


# ---------------------------------------------------------------------------
def _build_nc(win, sim_compat=False):
    nwt = win // 128
    nc = bass.Bass("TRN2", debug=False)

    def inp(name, shape, dt):
        return nc.dram_tensor(name, shape, dt, kind="ExternalInput").ap()

    # packed DRAM layouts: one DMA per logical group (long rows -> few,
    # large descriptors)
    wq_d = inp("wqp", (128, DT * D), F8)      # [p, di*512+j]
    hqT_d = inp("hqTp", (128, DT * RQ), F8)   # [p, di*768+q]
    wk_d = inp("wkp", (128, DT * D), F8)
    hkT_d = inp("hkTp", (128, DT * KC), F8)
    binvq_d = inp("binvq", (128, RQ), BF16)
    binvk_d = inp("binvk", (128, KC), BF16)
    bias_d = inp("biasp", (128, QT * win), BF16)  # [p, t*win+j]
    wv_d = inp("wvp", (128, DT * D), F8)
    bbv_d = inp("bbv", (128, D), F32)
    wo_d = inp("wop", (128, DT * D), BF16)
    xq_d = inp("xqp", (128, QT * D), BF16)      # x + bo, [p, t*512+j]
    bq_d = inp("bqp", (128, DT), F32)
    bk_d = inp("bkp", (128, DT), F32)
    w1_d = inp("w1p", (128, DT * F1), F8)
    b1_d = inp("b1p", (128, FT), F32)
    w2_d = inp("w2p", (128, FT * D), F8)
    bb2_d = inp("bb2", (128, D), F32)
    out_d = nc.dram_tensor("out", (RQ, D), F32, kind="ExternalOutput").ap()

    with tile.TileContext(nc) as tc:
        with tc.tile_pool(name="cst", bufs=1) as cst, \
             tc.tile_pool(name="wrk", bufs=2) as wrk, \
             tc.tile_pool(name="sml", bufs=3) as sml, \
             tc.tile_pool(name="acc", bufs=3, space="PSUM") as accp, \
             tc.tile_pool(name="tpp", bufs=3, space="PSUM") as tpp:

            def load(name, free, dt, src):
                t = cst.tile([128, free], dt, name=name, tag=name)
                nc.sync.dma_start(t[:], src)
                return t

            # loads in priority order (attention-critical first); hkT split
            # so kT matmuls on the first half start before the rest arrives
            # h-side tensors load on the ACT HWDGE ring, weights on the SP
            # ring — parallel descriptor issue halves the head DMA latency
            wq = load("wq", DT * D, F8, wq_d[:])
            hqT = cst.tile([128, DT * RQ], F8, name="hqT", tag="hqT")
            nc.scalar.dma_start(hqT[:], hqT_d[:])
            wk = load("wk", DT * D, F8, wk_d[:])
            hkTp = [cst.tile([128, 2 * KC], F8, name=f"hkT{i}", tag=f"hkT{i}")
                    for i in range(2)]
            nc.scalar.dma_start(hkTp[0][:], hkT_d[:, :2 * KC])
            wv = load("wv", DT * D, F8, wv_d[:])
            nc.scalar.dma_start(hkTp[1][:], hkT_d[:, 2 * KC:])
            binvq = load("binvq", RQ, BF16, binvq_d[:])
            binvk = load("binvk", KC, BF16, binvk_d[:])
            bias_all = load("bias_all", QT * win, BF16, bias_d[:])
            bbv = load("bbv", D, F32, bbv_d[:])
            wo = load("wo", DT * D, BF16, wo_d[:])
            xqb = load("xqb", QT * D, BF16, xq_d[:])
            bqp = load("bqp", DT, F32, bq_d[:])
            bkp = load("bkp", DT, F32, bk_d[:])
            w1 = cst.tile([128, DT * F1], F8, name="w1", tag="w1")
            _w1_dma = nc.sync.dma_start(w1[:], w1_d[:])
            b1p = load("b1p", FT, F32, b1_d[:])
            w2 = cst.tile([128, FT * D], F8, name="w2", tag="w2")
            _w2_dma = nc.sync.dma_start(w2[:], w2_d[:])
            bb2 = load("bb2", D, F32, bb2_d[:])

            epsc = cst.tile([128, 1], F32, name="epsc", tag="epsc")
            nc.vector.memset(epsc[:], LN_EPS)
            identb = cst.tile([128, 128], BF16, name="identb", tag="identb")
            make_identity(nc, identb[:])

            def wsl(t_, i, width):  # slice i of a packed [128, n*width] tile
                return t_[:, i * width:(i + 1) * width]

            def pair(t_, i2, width):  # [128, 2, width] DoubleRow view of pair i2
                return t_[:, i2 * 2 * width:(i2 + 1) * 2 * width].rearrange(
                    "p (two m) -> p two m", two=2)

            def hk_pair(i2):
                return hkTp[i2][:].rearrange("p (two m) -> p two m", two=2)

            def hk_sl(i):
                return hkTp[i // 2][:, (i % 2) * KC:((i % 2) + 1) * KC]

            # ---- z = h * invn ---------------------------------------------
            zqT = cst.tile([128, DT * RQ], F8, name="zqT", tag="zqT")
            zkT = cst.tile([128, DT * KC], F8, name="zkT", tag="zkT")
            for i in range(DT):
                nc.vector.tensor_tensor(wsl(zqT, i, RQ), wsl(hqT, i, RQ),
                                        binvq[:], ALU.mult)
                nc.vector.tensor_tensor(wsl(zkT, i, KC), hk_sl(i),
                                        binvk[:], ALU.mult)

            # ---- projections ----------------------------------------------
            qT = cst.tile([128, DT * RQ], BF16, name="qT", tag="qT")
            kTt = cst.tile([128, DT * KC], BF16, name="kTt", tag="kTt")
            vv = cst.tile([128, KT * D], BF16, name="vv", tag="vv")

            for do in range(DT):
                for qc in range(RQ // QCH):
                    ps = accp.tile([128, 512], F32, name="mmacc", tag="mmacc")
                    for di2 in range(DT // 2):
                        nc.tensor.matmul(
                            ps[:, :QCH],
                            lhsT=pair(wq, di2, D)[:, :, do * 128:(do + 1) * 128],
                            rhs=pair(hqT, di2, RQ)[:, :, qc * QCH:(qc + 1) * QCH],
                            start=(di2 == 0), stop=(di2 == DT // 2 - 1),
                            perf_mode=DR)
                    nc.scalar.activation(
                        wsl(qT, do, RQ)[:, qc * QCH:(qc + 1) * QCH], ps[:, :QCH],
                        AF.Identity, bias=bqp[:, do:do + 1], scale=1.0 / 16)
                for c0, cw in ((0, 512), (512, 512), (1024, 256)):
                    ps = accp.tile([128, 512], F32, name="mmacc", tag="mmacc")
                    for di2 in range(DT // 2):
                        nc.tensor.matmul(
                            ps[:, :cw],
                            lhsT=pair(wk, di2, D)[:, :, do * 128:(do + 1) * 128],
                            rhs=hk_pair(di2)[:, :, c0:c0 + cw],
                            start=(di2 == 0), stop=(di2 == DT // 2 - 1),
                            perf_mode=DR)
                    nc.scalar.activation(
                        wsl(kTt, do, KC)[:, c0:c0 + cw], ps[:, :cw],
                        AF.Identity, bias=bkp[:, do:do + 1], scale=1.0 / 16)

            for kt in range(KT):
                ps = accp.tile([128, 512], F32, name="mmacc", tag="mmacc")
                for di2 in range(DT // 2):
                    nc.tensor.matmul(
                        ps[:],
                        lhsT=hk_pair(di2)[:, :, kt * 128:(kt + 1) * 128],
                        rhs=pair(wv, di2, D),
                        start=(di2 == 0), stop=(di2 == DT // 2 - 1),
                        perf_mode=DR)
                _v_evict = nc.vector.scalar_tensor_tensor(
                    wsl(vv, kt, D), in0=ps[:], scalar=1.0 / 16, in1=bbv[:],
                    op0=ALU.mult, op1=ALU.add)

            # defer the 3MB of FFN weights until the attention-critical loads
            # and stage-1 are underway (head DMA bandwidth is the bottleneck)
            for _dma in (_w1_dma, _w2_dma):
                add_dep_helper(_dma.ins, _v_evict.ins, sync=True,
                               reason="defer FFN weight loads")

            # ---- attention, software-pipelined ----------------------------
            x2_all = cst.tile([128, QT * D], F32, name="x2_all", tag="x2_all")
            ss_all = cst.tile([128, QT], F32, name="ss_all", tag="ss_all")
            sq_all = cst.tile([128, QT], F32, name="sq_all", tag="sq_all")
            pn_q = {}

            def stage_a(t):
                woff = _woff(t, win)
                s_sb = wrk.tile([128, win], F32, name="s_sb", tag="s_sb")
                for c0 in range(0, win, 512):
                    cw = min(512, win - c0)
                    ps = accp.tile([128, 512], F32, name="s_ps", tag="mmacc")
                    # qk in bf16 (fp8 storage of q/k costs too much logit
                    # precision); the z-gram pairs run fp8 DoubleRow
                    nmm = DT + DT // 2
                    for j in range(DT):
                        nc.tensor.matmul(
                            ps[:, :cw],
                            lhsT=wsl(qT, j, RQ)[:, t * 128:(t + 1) * 128],
                            rhs=wsl(kTt, j, KC)[:, woff + c0:woff + c0 + cw],
                            start=(j == 0), stop=False)
                    for i in range(DT // 2):
                        j = DT + i
                        nc.tensor.matmul(
                            ps[:, :cw],
                            lhsT=pair(zqT, i, RQ)[:, :, t * 128:(t + 1) * 128],
                            rhs=pair(zkT, i, KC)[:, :, woff + c0:woff + c0 + cw],
                            start=False, stop=(j == nmm - 1),
                            perf_mode=DR)
                    # psum = sqrt(D)*(scale*qk + gram); normalize + add bias
                    nc.vector.scalar_tensor_tensor(
                        s_sb[:, c0:c0 + cw], in0=ps[:, :cw],
                        scalar=float(D) ** -0.5,
                        in1=bias_all[:, t * win + c0:t * win + c0 + cw],
                        op0=ALU.mult, op1=ALU.add)

                negm = sml.tile([128, 1], F32, name="negm", tag="negm")
                nc.vector.tensor_reduce(negm[:], s_sb[:], AX.X, ALU.max, negate=True)
                p_sb = wrk.tile([128, win], BF16, name="p_sb", tag="p_sb")
                den = sml.tile([128, 1], F32, name="den", tag="den")
                nc.scalar.activation(p_sb[:], s_sb[:], AF.Exp,
                                     bias=negm[:], scale=1.0, accum_out=den[:])
                rr = sml.tile([128, 1], F32, name="rr", tag="rr")
                nc.vector.reciprocal(rr[:], den[:])
                pn = wrk.tile([128, win], BF16, name="pn", tag="pn", bufs=3)
                nc.vector.tensor_scalar(pn[:], p_sb[:], rr[:], None, ALU.mult)
                pn_q[t] = pn

            def stage_b(t):
                woff = _woff(t, win)
                pn = pn_q.pop(t)
                pT = wrk.tile([128, win], BF16, name="pT", tag="pT")
                for j in range(nwt):
                    tp = tpp.tile([128, 128], BF16, name="tp_ps", tag="tp_ps")
                    nc.tensor.transpose(tp[:], pn[:, j * 128:(j + 1) * 128],
                                        identb[:])
                    nc.vector.tensor_copy(pT[:, j * 128:(j + 1) * 128], tp[:])

                # oT[d,q] = v^T @ p^T (no output transpose needed)
                oT = wrk.tile([128, D], BF16, name="oT", tag="oT")
                for dt_ in range(DT):
                    o_ps = tpp.tile([128, 128], F32, name="o_ps", tag="o_ps", bufs=2)
                    for j in range(nwt):
                        nc.tensor.matmul(
                            o_ps[:],
                            lhsT=wsl(vv, woff // 128 + j, D)[:, dt_ * 128:(dt_ + 1) * 128],
                            rhs=pT[:, j * 128:(j + 1) * 128],
                            start=(j == 0), stop=(j == nwt - 1))
                    nc.scalar.copy(oT[:, dt_ * 128:(dt_ + 1) * 128], o_ps[:])

                x2_ps = accp.tile([128, 512], F32, name="x2_ps", tag="mmacc")
                for dt_ in range(DT):
                    nc.tensor.matmul(
                        x2_ps[:],
                        lhsT=oT[:, dt_ * 128:(dt_ + 1) * 128],
                        rhs=wsl(wo, dt_, D),
                        start=(dt_ == 0), stop=(dt_ == DT - 1))
                x2 = wsl(x2_all, t, D)
                nc.vector.tensor_tensor(x2, x2_ps[:], wsl(xqb, t, D), ALU.add)

                # LN2 stats (scale/var finished in batched epilogue)
                nc.vector.reduce_sum(ss_all[:, t:t + 1], x2, AX.X)
                sq_sc = wrk.tile([128, D], F32, name="sq_sc", tag="sq_sc")
                nc.vector.tensor_tensor(sq_sc[:], x2, x2, ALU.mult)
                nc.vector.reduce_sum(sq_all[:, t:t + 1], sq_sc[:], AX.X)

            # LN2 finish + xn2 + transposes for a half (3 q-tiles)
            xn2T_h = [cst.tile([128, DT * QCH], F8, name=f"xn2T{h}",
                               tag=f"xn2T{h}") for h in range(2)]
            rstd_h = [None, None]
            nmr_h = [None, None]

            def ln2_finish(h):
                sl = slice(h * 3, h * 3 + 3)
                mu = sml.tile([128, 3], F32, name="mu3", tag="mu3")
                nc.vector.tensor_scalar(mu[:], ss_all[:, sl], 1.0 / D, None, ALU.mult)
                msq = sml.tile([128, 3], F32, name="msq3", tag="msq3")
                nc.vector.tensor_scalar(msq[:], sq_all[:, sl], 1.0 / D, None, ALU.mult)
                mu2 = sml.tile([128, 3], F32, name="mu23", tag="mu23")
                nc.vector.tensor_tensor(mu2[:], mu[:], mu[:], ALU.mult)
                nv = sml.tile([128, 3], F32, name="nv3", tag="nv3")
                nc.vector.tensor_tensor(nv[:], mu2[:], msq[:], ALU.subtract)
                sd = sml.tile([128, 3], F32, name="sd3", tag="sd3")
                nc.scalar.activation(sd[:], nv[:], AF.Sqrt, bias=epsc[:], scale=-1.0)
                rstd = cst.tile([128, 3], F32, name=f"rstd{h}", tag=f"rstd{h}")
                nc.vector.reciprocal(rstd[:], sd[:])
                nmr = cst.tile([128, 3], F32, name=f"nmr{h}", tag=f"nmr{h}")
                nc.vector.scalar_tensor_tensor(
                    nmr[:], in0=mu[:], scalar=-1.0, in1=rstd[:],
                    op0=ALU.mult, op1=ALU.mult)
                rstd_h[h], nmr_h[h] = rstd, nmr

            def xn2_tp(t):
                h, pos = divmod(t, 3)
                xn2 = wrk.tile([128, D], BF16, name="xn2", tag="xn2")
                nc.vector.tensor_scalar(
                    xn2[:], wsl(x2_all, t, D), rstd_h[h][:, pos:pos + 1],
                    nmr_h[h][:, pos:pos + 1], ALU.mult, ALU.add)
                for dt_ in range(DT):
                    tp = tpp.tile([128, 128], BF16, name="tp_ps", tag="tp_ps")
                    nc.tensor.transpose(tp[:], xn2[:, dt_ * 128:(dt_ + 1) * 128],
                                        identb[:])
                    nc.scalar.copy(
                        wsl(xn2T_h[h], dt_, QCH)[:, pos * 128:(pos + 1) * 128], tp[:])

            g_h = [cst.tile([128, FT * QCH], F8, name=f"g{h}", tag=f"g{h}")
                   for h in range(2)]

            def ffn1(h):
                for ft in range(FT):
                    ps = accp.tile([128, 512], F32, name="a_ps", tag="mmacc")
                    for di2 in range(DT // 2):
                        nc.tensor.matmul(
                            ps[:, :QCH],
                            lhsT=pair(w1, di2, F1)[:, :, ft * 128:(ft + 1) * 128],
                            rhs=pair(xn2T_h[h], di2, QCH),
                            start=(di2 == 0), stop=(di2 == DT // 2 - 1),
                            perf_mode=DR)
                    gsl = wsl(g_h[h], ft, QCH)
                    if sim_compat:
                        t1 = wrk.tile([128, QCH], F32, name="g_t1", tag="g_t1")
                        nc.scalar.activation(t1[:], ps[:, :QCH], AF.Identity,
                                             bias=b1p[:, ft:ft + 1], scale=1.0 / 16)
                        t2 = wrk.tile([128, QCH], F32, name="g_t2", tag="g_t2")
                        nc.scalar.activation(t2[:], t1[:], AF.Sigmoid,
                                             bias=0.0, scale=1.702)
                        nc.vector.tensor_tensor(gsl, t1[:], t2[:], ALU.mult)
                    else:
                        nc.scalar.activation(gsl, ps[:, :QCH], AF.Gelu,
                                             bias=b1p[:, ft:ft + 1], scale=1.0 / 16)

            def ffn2(t):
                h, pos = divmod(t, 3)
                f_ps = accp.tile([128, 512], F32, name="f_ps", tag="mmacc")
                for ft2 in range(FT // 2):
                    nc.tensor.matmul(
                        f_ps[:],
                        lhsT=pair(g_h[h], ft2, QCH)[:, :, pos * 128:(pos + 1) * 128],
                        rhs=pair(w2, ft2, D),
                        start=(ft2 == 0), stop=(ft2 == FT // 2 - 1),
                        perf_mode=DR)
                ob = wrk.tile([128, D], F32, name="ob", tag="ob")
                nc.vector.scalar_tensor_tensor(
                    ob[:], in0=f_ps[:], scalar=1.0 / 16, in1=bb2[:],
                    op0=ALU.mult, op1=ALU.add)
                nc.vector.tensor_tensor(ob[:], ob[:], wsl(x2_all, t, D), ALU.add)
                nc.sync.dma_start(out_d[t * 128:(t + 1) * 128, :], ob[:])

            # pipeline: S(t) ahead of consume(t-1); first-half LN2 + its
            # transposes run during the second half of attention
            for t in range(QT):
                stage_a(t)
                if t == 5:
                    for tt_ in (0, 1, 2):
                        xn2_tp(tt_)
                if t >= 2:
                    stage_b(t - 2)
                if t == 4:
                    ln2_finish(0)
            stage_b(QT - 2)
            stage_b(QT - 1)
            ln2_finish(1)
            ffn1(0)
            for tt_ in (3, 4, 5):
                xn2_tp(tt_)
            for t in (0, 1, 2):
                ffn2(t)
            ffn1(1)
            for t in (3, 4, 5):
                ffn2(t)

    return nc


_NC_CACHE = {}


def _get_nc(win, sim_compat=False):
    key = (win, sim_compat)
    if key not in _NC_CACHE:
        _NC_CACHE[key] = _build_nc(win, sim_compat)
    return _NC_CACHE[key]


def _pack(a, parts, width):
    """(parts*128, width) -> (128, parts*width) with [p, i*width+j] = a[i*128+p, j]."""
    return np.ascontiguousarray(
        a.reshape(parts, 128, width).transpose(1, 0, 2).reshape(128, parts * width))


# ---------------------------------------------------------------------------
def _host_prep(x, grid, Wq, bq, Wk, bk, Wv, bv, Wo, bo,
               ln1_g, ln1_b, ln2_g, ln2_b, W1, b1, W2, b2):
    f32 = np.float32
    bf16 = ml_dtypes.bfloat16
    x = np.ascontiguousarray(np.asarray(x, f32))
    grid = np.asarray(grid)
    gx = grid[:, 0].astype(np.int64)
    gy = grid[:, 1].astype(np.int64)
    assert x.shape == (N, D)

    perm = np.lexsort((gy, gx))
    xs = x[perm]
    gxs = gx[perm]
    gys = gy[perm]

    mu = xs.mean(axis=1, keepdims=True, dtype=f32)
    var = xs.var(axis=1, keepdims=True, dtype=f32)
    h = ((xs - mu) / np.sqrt(var + LN_EPS)) * np.asarray(ln1_g, f32) \
        + np.asarray(ln1_b, f32)
    h = h.astype(f32)
    invn = (1.0 / np.maximum(np.linalg.norm(h, axis=1), COS_EPS)).astype(f32)

    hpad = np.zeros((N + 2 * PAD, D), f32)
    hpad[PAD:PAD + N] = h
    invnpad = np.ones(N + 2 * PAD, f32)
    invnpad[PAD:PAD + N] = invn
    gxpad = np.full(N + 2 * PAD, -(10 ** 6), np.int64)
    gxpad[PAD:PAD + N] = gxs
    gxpad[PAD + N:] = 10 ** 6
    gypad = np.zeros(N + 2 * PAD, np.int64)
    gypad[PAD:PAD + N] = gys

    win = None
    lo_cs = None
    for cand in WIN_LADDER:
        lo_try = []
        ok = True
        for c in range(NCORES):
            q0 = c * RQ
            lo_min, lo_max = -PAD, N + PAD
            for t in range(QT):
                a, b_ = q0 + t * 128, q0 + (t + 1) * 128
                w_ = _woff(t, cand)
                lo_t = int(np.searchsorted(gxs, gxs[a] - RADIUS, "left"))
                hi_t = int(np.searchsorted(gxs, gxs[b_ - 1] + RADIUS, "right"))
                lo_min = max(lo_min, hi_t - w_ - cand)
                lo_max = min(lo_max, lo_t - w_)
                lo_min = max(lo_min, q0 + t * 128 - w_ - (cand - 128))
                lo_max = min(lo_max, q0 + t * 128 - w_)
            lo_min = max(lo_min, -PAD)
            lo_max = min(lo_max, N + PAD - KC)
            if lo_min > lo_max:
                ok = False
                break
            lo_try.append((lo_min + lo_max) // 2)
        if ok:
            win, lo_cs = cand, lo_try
            break
    assert win is not None, "no feasible static window schedule"

    # fp8 operands: weights x16 on host, 1/16 at PSUM eviction; q/k kept at
    # natural scale (alpha=1); z-side scaled by D**0.25 so the fused S chain
    # computes sqrt(D)*(qk/sqrt(D) + gram), normalized at S eviction.
    fp8 = ml_dtypes.float8_e4m3
    wq_p = _pack((np.asarray(Wq, f32) * 16).astype(fp8), DT, D)
    wk_p = _pack((np.asarray(Wk, f32) * 16).astype(fp8), DT, D)
    wv_p = _pack((np.asarray(Wv, f32) * 16).astype(fp8), DT, D)
    wo_p = _pack(np.asarray(Wo, f32).astype(bf16), DT, D)
    w2_p = _pack((np.asarray(W2, f32) * 16).astype(fp8), FT, D)

    bqp = np.ascontiguousarray(np.asarray(bq, f32).reshape(DT, 128).T)
    bkp = np.ascontiguousarray(np.asarray(bk, f32).reshape(DT, 128).T)
    bbv = np.broadcast_to(np.asarray(bv, f32), (128, D)).copy()
    bb2 = np.broadcast_to(np.asarray(b2, f32), (128, D)).copy()

    g2 = np.asarray(ln2_g, f32)
    b2g = np.asarray(ln2_b, f32)
    w1_p = _pack((16 * g2[:, None] * np.asarray(W1, f32)).astype(fp8), DT, F1)
    b1_eff = b2g @ np.asarray(W1, f32) + np.asarray(b1, f32)
    b1p = np.ascontiguousarray(b1_eff.reshape(FT, 128).T.astype(f32))
    zscale = f32(float(D) ** 0.25)

    bo_f = np.asarray(bo, f32)

    in_maps = []
    for c in range(NCORES):
        q0 = c * RQ
        lo = lo_cs[c]
        plo = lo + PAD
        hk = hpad[plo:plo + KC]
        invk = invnpad[plo:plo + KC]
        gxk = gxpad[plo:plo + KC]
        gyk = gypad[plo:plo + KC]

        bias_m = np.full((RQ, win), f32(NEGINF), f32)
        for t in range(QT):
            w_ = _woff(t, win)
            m = (np.abs(gxs[q0 + t * 128:q0 + (t + 1) * 128, None]
                        - gxk[None, w_:w_ + win]) <= RADIUS) & \
                (np.abs(gys[q0 + t * 128:q0 + (t + 1) * 128, None]
                        - gyk[None, w_:w_ + win]) <= RADIUS)
            selfpos = (q0 + t * 128 + np.arange(128)) - lo - w_
            assert (selfpos >= 0).all() and (selfpos < win).all()
            m[np.arange(128), selfpos] = False
            bm = np.where(m, f32(0.0), f32(NEGINF))
            iso = ~m.any(axis=1)
            if iso.any():
                bm[np.nonzero(iso)[0], selfpos[iso]] = 0.0
            bias_m[t * 128:(t + 1) * 128] = bm

        hq = h[q0:q0 + RQ]
        in_maps.append({
            "wqp": wq_p,
            "hqTp": _pack(np.ascontiguousarray(hq.T).astype(fp8), DT, RQ),
            "wkp": wk_p,
            "hkTp": _pack(np.ascontiguousarray(hk.T).astype(fp8), DT, KC),
            "binvq": np.broadcast_to(invn[q0:q0 + RQ] * zscale,
                                     (128, RQ)).astype(bf16),
            "binvk": np.broadcast_to(invk * zscale, (128, KC)).astype(bf16),
            "biasp": _pack(bias_m.astype(bf16), QT, win),
            "wvp": wv_p, "bbv": bbv, "wop": wo_p,
            "xqp": _pack((xs[q0:q0 + RQ] + bo_f).astype(bf16), QT, D),
            "bqp": bqp, "bkp": bkp,
            "w1p": w1_p, "b1p": b1p, "w2p": w2_p, "bb2": bb2,
        })
    return in_maps, perm, win


def kernel(x, grid, Wq, bq, Wk, bk, Wv, bv, Wo, bo,
           ln1_g, ln1_b, ln2_g, ln2_b, W1, b1, W2, b2):
    global LAST_EXEC_NS, LAST_RESULTS
    in_maps, perm, win = _host_prep(x, grid, Wq, bq, Wk, bk, Wv, bv, Wo, bo,
                                    ln1_g, ln1_b, ln2_g, ln2_b, W1, b1, W2, b2)
    nc = _get_nc(win, sim_compat=False)
    trace = os.environ.get("BASS_KERNEL_TRACE", "0") == "1"
    kw = {}
    if trace:
        _install_ntff_hook()
        kw = dict(trace=True, tmpdir=os.environ.get("BASS_KERNEL_TRACE_DIR"))
    res = bass_utils.run_bass_kernel_spmd(
        nc, in_maps, core_ids=list(range(NCORES)), **kw)
    LAST_EXEC_NS = res.exec_time_ns
    LAST_RESULTS = res
    out = np.empty((N, D), np.float32)
    for c in range(NCORES):
        out[perm[c * RQ:(c + 1) * RQ]] = res.results[c]["out"]
    return out
